# revision 8
# baseline (speedup 1.0000x reference)
"""Trainium2 Bass kernel for nn_AttentionNet (spatial-attention net).

Math restructure (host-side fold of the small projection weights):
    f = feat.reshape(B, C, N)                       N = 14*14 = 196
    query = w2v @ Wq + bq                           [S, M]
    scores[b,s,n] = (query Wk^T) @ f_b + const(s)   softmax over n drops const
    Qk = query @ Wk^T                               [S, C]
    U  = V @ Wo^T ; P = U @ Wv^T                    [S, C]
    attended term  = sum_n softmax(Qk@f_b)[s,n] * (P@f_b)[s,n]
    pool+bias term = HOST-precomputed: pk[b,s] = mean_n(f_b) @ V[s,:] + kc[s]
    v2s[b,s] = attended + pk

Device work per core (16 of 128 batches, data parallel over 8 cores):
    All PE operands in fp16 (full PE rate, half the HBM traffic of f32r,
    FastWeightLoad active so LDWEIGHTS never paces the matmul stream).
    Per batch-pair: 5 column-groups x 16 K-chunks of [128xm]@[128x392]
    matmuls, softmax on ACT/DVE straight out of PSUM (joint pair-max so a
    single exp covers both batches), fused multiply+reduce for the
    attended numerator. Final: 3 PE transposes + DVE add of pk + DMA out.
"""

import numpy as np

import concourse.bass as bass
import concourse.tile as tile
from concourse import mybir
from concourse.bass_utils import run_bass_kernel_spmd
from concourse.masks import make_identity

B, C, N = 128, 2048, 196
S = 312
NCORES = 8
BL = B // NCORES            # batches per core
NPAIR = BL // 2             # batch pairs per core (2 batches share a matmul)
CCH = C // 128              # contraction chunks
SCHUNKS = [(0, 128), (128, 128), (256, 56)]
F32 = mybir.dt.float32
F16 = mybir.dt.float16
AX = mybir.AxisListType
ALU = mybir.AluOpType
ACTF = mybir.ActivationFunctionType
WARMUP = 6                  # junk matmuls to warm the PE clock during DMA wait

_NC = None
_RESULTS = None  # last BassKernelResults, for profiling harnesses


def _build_kernel():
    nc = bass.Bass("TRN2", debug=False, target_bir_lowering=False,
                   num_devices=NCORES)
    feat = nc.dram_tensor("feat", [128, NPAIR * CCH * 392], F16,
                          kind="ExternalInput").ap()
    qpt = nc.dram_tensor("qpt", [128, CCH * 632], F16, kind="ExternalInput").ap()
    pk = nc.dram_tensor("pk", [BL, S], F32, kind="ExternalInput").ap()
    v2s = nc.dram_tensor("v2s", [BL, S], F32, kind="ExternalOutput").ap()

    fr = feat.rearrange("p (pr k m) -> p pr k m", pr=NPAIR, k=CCH)
    qpr = qpt.rearrange("p (k s) -> p k s", s=632)

    with tile.TileContext(nc) as tc:
        from contextlib import ExitStack
        with ExitStack() as ctx:
            consts = ctx.enter_context(tc.tile_pool(name="consts", bufs=1))
            fpool = ctx.enter_context(tc.tile_pool(name="f", bufs=3))
            epool = ctx.enter_context(tc.tile_pool(name="e", bufs=3))
            prpool = ctx.enter_context(tc.tile_pool(name="prod", bufs=3))
            spool = ctx.enter_context(tc.tile_pool(name="small", bufs=12))
            pss = ctx.enter_context(tc.tile_pool(name="pss", bufs=3, space="PSUM"))
            psw = ctx.enter_context(tc.tile_pool(name="psw", bufs=3, space="PSUM"))
            psout = ctx.enter_context(tc.tile_pool(name="psout", bufs=1, space="PSUM"))
            psjunk = ctx.enter_context(tc.tile_pool(name="psjunk", bufs=1, space="PSUM"))

            junk = psjunk.tile([128, 128], F32)

            # Persistent SBUF state.  Packed weight columns per c-chunk:
            # [Qk s0 | Qk s1 | P s0 | P s1 | Qk s2 | pad8 | P s2] so every
            # matmul group is one contiguous block.
            qp_sb = consts.tile([128, CCH, 632], F16)
            ident = consts.tile([128, 128], F32)
            # per-s-chunk result rows [s_part, b]; separate tiles so each
            # transpose depends only on its own chunk's softmax writes
            term2 = [consts.tile([128, BL], F32, name=f"term2_{i}")
                     for i in range(3)]
            pk_sb = consts.tile([BL, S], F32)

            make_identity(nc, ident)

            # First weight chunks + pool/bias term on the ring up front.
            nc.sync.dma_start(out=qp_sb[:, 0:2], in_=qpr[:, 0:2])
            nc.sync.dma_start(out=pk_sb[:], in_=pk)

            # Warm the PE clock while the first DMAs land: full 128-partition
            # fp32 matmuls are 4 cycles/row -> ~430ns each at the cold clock,
            # so ~8 of them cover one HAM activity window and the real matmul
            # stream starts at 2.4 GHz.
            for _ in range(WARMUP):
                nc.tensor.matmul(junk[:], ident[:], ident[:],
                                 start=True, stop=True)

            def softmax_stage(scores_ps, w_ps, m, sc, pr):
                # scores_ps/w_ps: [m, 2, N] PSUM APs (may live in one tile at
                # different partition offsets for the packed tail chunk).
                negmax = spool.tile([m, 2], F32, tag="negmax")
                nc.vector.reduce_max(out=negmax, in_=scores_ps, axis=AX.X,
                                     negate=True)
                e = epool.tile([m, 2, N], F16, tag="e")
                for h in range(2):
                    nc.scalar.activation(out=e[:, h, :], in_=scores_ps[:, h, :],
                                         func=ACTF.Exp,
                                         bias=negmax[:, h:h + 1], scale=1.0)
                den = spool.tile([m, 2], F32, tag="den")
                nc.vector.reduce_sum(out=den, in_=e[:], axis=AX.X)
                prod = prpool.tile([m, 2, N], F32, tag="prod")
                nc.vector.tensor_mul(out=prod[:], in0=e[:], in1=w_ps)
                num = spool.tile([m, 2], F32, tag="num")
                nc.vector.reduce_sum(out=num, in_=prod[:], axis=AX.X)
                rcp = spool.tile([m, 2], F32, tag="rcp")
                nc.vector.reciprocal(rcp, den[:])
                nc.vector.tensor_mul(
                    out=term2[sc][0:m, 2 * pr:2 * pr + 2],
                    in0=num[:], in1=rcp[:],
                )

            f1_prefetch = None
            for pr in range(NPAIR):
                if pr == 0:
                    # Interleave the remaining weight chunks with pair-0 feat
                    # slices at 2-chunk granularity so the c-major matmuls of
                    # pair 0 pace with ring delivery (each step delivers one
                    # ~0.5 MB qp+feat slice vs ~1.6us of PE consumption).
                    f_tile = fpool.tile([128, CCH, 2, N], F16, name="f0", tag="f")
                    for j in range(8):
                        nc.sync.dma_start(out=f_tile[:, 2 * j:2 * j + 2],
                                          in_=fr[:, 0, 2 * j:2 * j + 2])
                        if j < 7:
                            c0 = 2 + 2 * j
                            nc.sync.dma_start(out=qp_sb[:, c0:c0 + 2],
                                              in_=qpr[:, c0:c0 + 2])
                    f1_prefetch = fpool.tile([128, CCH, 2, N], F16, name="f1",
                                             tag="f")
                    for q in range(4):
                        nc.sync.dma_start(out=f1_prefetch[:, 4 * q:4 * q + 4],
                                          in_=fr[:, 1, 4 * q:4 * q + 4])
                elif pr == 1:
                    f_tile = f1_prefetch
                else:
                    f_tile = fpool.tile([128, CCH, 2, N], F16, name="fx", tag="f")
                    nc.sync.dma_start(out=f_tile[:], in_=fr[:, pr])

                # Column blocks of the packed weights: (psum rows, col0)
                groups = [(128, 0), (128, 256), (128, 128), (128, 384), (120, 512)]
                tiles = []
                for gi, (m, c0) in enumerate(groups):
                    pool = psw if gi in (1, 3) else pss
                    tiles.append(pool.tile([m, 2, N], F32, name=f"psg{gi}",
                                           tag="psw" if gi in (1, 3) else "pss"))
                if pr <= 1:
                    # c-major: consume weight/feat chunks as the DMAs land.
                    for ck in range(CCH):
                        for gi, (m, c0) in enumerate(groups):
                            nc.tensor.matmul(
                                tiles[gi][:], qp_sb[:, ck, c0:c0 + m],
                                f_tile[:, ck],
                                start=(ck == 0), stop=(ck == CCH - 1),
                            )
                else:
                    for gi, (m, c0) in enumerate(groups):
                        for ck in range(CCH):
                            nc.tensor.matmul(
                                tiles[gi][:], qp_sb[:, ck, c0:c0 + m],
                                f_tile[:, ck],
                                start=(ck == 0), stop=(ck == CCH - 1),
                            )
                softmax_stage(tiles[0][:], tiles[1][:], 128, 0, pr)
                softmax_stage(tiles[2][:], tiles[3][:], 128, 1, pr)
                softmax_stage(tiles[4][0:56], tiles[4][64:120], 56, 2, pr)

            # Final: out[b, s] = term2^T (PE transposes) + pk (DVE add).
            out_ps = psout.tile([BL, S], F32)
            for sc, (s0, m) in enumerate(SCHUNKS):
                nc.tensor.matmul(
                    out_ps[:, s0:s0 + m],
                    term2[sc][0:m, :],
                    ident[0:m, 0:m],
                    is_transpose=True,
                    start=(sc == 0), stop=(sc == 2),
                )
            final_sb = consts.tile([BL, S], F32)
            nc.vector.tensor_add(out=final_sb[:], in0=out_ps[:], in1=pk_sb[:])
            nc.sync.dma_start(out=v2s, in_=final_sb[:])

    _strip_pe_self_waits(nc)
    _hoist_excess_waits(nc)
    return nc


def _strip_pe_self_waits(nc):
    """Remove PE-on-PE semaphore waits from PE instructions.

    Tile's PSUM slot-reuse release emits a wait on the PE engine's own
    semaphore alongside the cross-engine reader wait. The self-wait can never
    guard a real hazard (PE reads only SBUF, writes only PSUM, and retires
    writes in order), and walrus allows only one sync wait per instruction.
    """
    def walk(b):
        for i in getattr(b, "instructions", []) or []:
            if str(getattr(i, "engine", "")).endswith("PE"):
                si = i.sync_info
                if si is not None and si.on_wait:
                    kept = [w for w in si.on_wait
                            if not str(w.ant_name).startswith("PE_")]
                    if len(kept) != len(si.on_wait):
                        si.on_wait = kept
        for sb in getattr(b, "blocks", []) or []:
            walk(sb)
    for b in nc.m.functions[0].blocks:
        walk(b)


def _hoist_excess_waits(nc):
    """Walrus allows a single sync wait per TPB instruction (one EVENTS slot).

    Tile sometimes emits 2+ waits on one instruction (e.g. a tile written by
    two DMAs, or a PSUM slot released by readers on two engines). Hoist all
    but one wait onto standalone EventSemaphore instructions inserted just
    before the consumer on the same engine - identical semantics, one wait
    per hardware instruction.
    """
    import bass_rust

    # Pick semaphore ids no instruction references (alloc_semaphore would
    # recycle ids of released-but-still-referenced Tile sems).
    used = set()
    for b in nc.m.functions[0].blocks:
        for i in b.instructions or []:
            si = i.sync_info
            if si is not None:
                for w in si.on_wait or []:
                    used.add(w.id)
                for u in si.on_update or []:
                    used.add(u.id)
    free = (i for i in range(255, -1, -1) if i not in used)
    sems = {}

    def sem_for(engine):
        key = str(engine)
        if key not in sems:
            sems[key] = (next(free), f"hoist_waits_{key.split('.')[-1]}")
        return sems[key]

    for b in nc.m.functions[0].blocks:
        insts = list(b.instructions or [])
        out = []
        changed = False
        for i in insts:
            si = i.sync_info
            waits = list(si.on_wait) if si is not None and si.on_wait else []
            if len(waits) > 1:
                for w in waits[:-1]:
                    ev = mybir.InstEventSemaphore(
                        name=f"hoist-{nc.next_id()}", ins=[], outs=[])
                    ev.engine = i.engine
                    # The update to a dedicated (never-waited) semaphore keeps
                    # CoreSim's event loop happy - every instruction must
                    # carry at least one sem update.
                    sem_id, sem_name = sem_for(i.engine)
                    upd = bass_rust.SyncUpdate(
                        sync_type="semaphore", id=sem_id, ant_name=sem_name,
                        update_mode="sem-inc", update_value=1)
                    ev.sync_info = bass_rust.SyncInfo(on_wait=[w], on_update=[upd])
                    out.append(ev)
                si.on_wait = [waits[-1]]
                changed = True
            out.append(i)
        if changed:
            b.instructions = out
    return nc


def _get_nc():
    global _NC
    if _NC is None:
        _NC = _build_kernel()
    return _NC


def _precompute(feat, w2v_att, Wq, bq, Wk, bk, Wv, bv, Wo, bo, V_att_final):
    d = lambda x: np.asarray(x, np.float64)
    query = d(w2v_att) @ d(Wq) + d(bq)              # [S, M]
    Qk = query @ d(Wk).T                            # [S, C]
    U = d(V_att_final) @ d(Wo).T                    # [S, M]
    P = U @ d(Wv).T                                 # [S, C]
    kc = U @ d(bv) + d(V_att_final) @ d(bo)         # [S]
    QkT, PT = Qk.T.astype(np.float16), P.T.astype(np.float16)
    # Tail block pads 8 zero columns so the P rows land on partition 64
    # (engine partition offsets must be 32-aligned).
    qpt = np.concatenate([QkT[:, 0:128], QkT[:, 128:256], PT[:, 0:128],
                          PT[:, 128:256], QkT[:, 256:312],
                          np.zeros((C, 8), np.float16), PT[:, 256:312]],
                         axis=1)                                  # [C, 632]
    # shuffle to [128, k*cols] so device loads are 128 contiguous descriptors
    qpt = np.ascontiguousarray(
        qpt.reshape(CCH, 128, 632).transpose(1, 0, 2).reshape(128, CCH * 632))

    f = np.asarray(feat, np.float32).reshape(B, C, N)
    # pool + attended-bias term, exact on host: pk[b,s] = mean_n f . V + kc
    pool = f.sum(axis=2, dtype=np.float64) / N                  # [B, C]
    pk = (pool @ d(V_att_final).T + kc[None, :]).astype(np.float32)  # [B, S]

    # feat device layout: per core [128, pair, chunk, 2*196] fp16 so every
    # DMA is 128 contiguous per-partition segments.
    fh = f.astype(np.float16).reshape(NCORES, BL, CCH, 128, N)
    fl = fh.transpose(0, 3, 1, 2, 4)                 # [core, p, b, ck, n]
    fl = fl.reshape(NCORES, 128, NPAIR, 2, CCH, N).transpose(0, 1, 2, 4, 3, 5)
    fl = np.ascontiguousarray(fl).reshape(NCORES, 128, NPAIR * CCH * 392)
    return fl, qpt, pk


def _ensure_ntff_hook():
    """If BASS_TRACE is set in the environment, run_bass_kernel_spmd imports
    antenv.axon_hooks, which this image lacks - graft the ctypes NTFF hook
    from trn_boot so tracing degrades gracefully instead of crashing."""
    import sys
    if "antenv.axon_hooks" in sys.modules:
        return
    try:
        import antenv.axon_hooks  # noqa: F401
    except ImportError:
        try:
            import types
            import trn_agent_boot.trn_boot as tb
            hook = tb._ntff_profile_via_ctypes("/opt/axon/libaxon_pjrt.so")
            m = types.ModuleType("antenv.axon_hooks")
            m.get_axon_ntff_profile_hook = lambda: hook
            sys.modules["antenv.axon_hooks"] = m
        except Exception:
            pass


def kernel(**inputs):
    global _RESULTS
    _ensure_ntff_hook()
    fl, qpt, pk = _precompute(
        inputs["feat"], inputs["w2v_att"], inputs["Wq"], inputs["bq"],
        inputs["Wk"], inputs["bk"], inputs["Wv"], inputs["bv"], inputs["Wo"],
        inputs["bo"], inputs["V_att_final"],
    )
    nc = _get_nc()
    in_maps = [
        {
            "feat": fl[core],
            "qpt": qpt,
            "pk": np.ascontiguousarray(pk[core * BL:(core + 1) * BL]),
        }
        for core in range(NCORES)
    ]
    _RESULTS = run_bass_kernel_spmd(nc, in_maps, core_ids=list(range(NCORES)))
    return np.concatenate([r["v2s"] for r in _RESULTS.results], axis=0)
